# revision 37
# baseline (speedup 1.0000x reference)
"""Trainium2 Bass kernel for nn_Colorcal_TwoDatasets (per-sample affine color
calibration with per-(cam,id,dataset) gathered scale/bias).

Contract: kernel(**inputs) takes the FULL unsharded inputs (see shapes below),
shards the batch across 8 NeuronCores (2 samples per core, pure data parallel),
runs a Bass/Tile kernel per core, and gathers the full [16,3,1024,1024] output.

Device kernel per core:
  - the (cam,id,dataset) gather runs on-device on 12 partitions (one per
    gathered scale/bias value): masked one-hot compares against an iota over
    the concatenated tables, one tensor_mul + tensor_reduce, then a tiny
    SBUF->SBUF transpose DMA + gpsimd partition_broadcast produce [128,12]
    per-partition scale/bias operands
  - the 24 MiB image shard is streamed plane-by-plane through SBUF (one 4 MiB
    HWDGE DMA per plane, triple-buffered) with one fused multiply-add per
    plane, alternating DVE tensor_scalar / ACT activation(Identity)
"""

import numpy as np

import concourse.bacc as bacc
import concourse.mybir as mybir
import concourse.tile as tile
from concourse import bass_utils

N_CORES = 8
B, C, H, W = 16, 3, 1024, 1024
BPC = B // N_CORES  # samples per core
NC1, NI1, NC2, NI2 = 40, 256, 80, 512
SEG = NC1 + NI1 + NC2 + NI2  # 888: [cam1 | ident1 | cam2 | ident2]
PF = H * W // 128  # 8192 free elements per plane per partition
TILE_F = 8192  # free-dim tile size: full plane per DMA (4 MiB), best HBM BW
F32 = mybir.dt.float32
F16 = mybir.dt.float16
I8 = mybir.dt.int8
# IO mode for the streamed image/output. The rel-err gate is 2e-2:
#   f16   — fp16 in / fp16 out  (rel err ~2.9e-4, 4 B/elem HBM traffic)
#   i8f16 — int8 in / fp16 out  (rel err ~9.1e-3, 3 B/elem)
#   i8i8  — int8 in / int8 out  (rel err ~1.4e-2, 2 B/elem)
# int8 uses per-(plane, partition-row-block) scales; dequant scale for the
# output is 1.15x the input scale so the affine result never clips.
MODE = "f16"
OUT_SCL = 1.10  # output int8 scale headroom over input scale (|w|<=1.04 +
                # |b|/(OUT_SCL*s) <= ~2 keeps |z| <= ~121 < 127: no clipping)
BUFS = 4  # io tile-pool depth

_CACHE = {}

_SEGS = (
    # (start, end, idx_col) over the concatenated [cam1|ident1|cam2|ident2] axis;
    # idx_col: 0=cam, 1=id; mask: 0 -> dataset==0 segment, 1 -> dataset==1
    (0, NC1, 0, 0),
    (NC1, NC1 + NI1, 1, 0),
    (NC1 + NI1, NC1 + NI1 + NC2, 0, 1),
    (NC1 + NI1 + NC2, SEG, 1, 1),
)


def _gather12(nc, cpool, spool, aux, wb_t, NR, dma_eng=None):
    """Gather on NR=12 partitions (one row per output value), then broadcast.
    Row r = off*6 + i*3 + c carries sample i(r)'s indices and the (w|b, c)
    table slice; one mul+reduce computes all 12 dot products at once.
    aux columns: [0:4) idx(cam,id,dt,-), [4:4+SEG) iota, [4+SEG:4+2*SEG) table."""
    if dma_eng is None:
        dma_eng = nc.sync
    mult = mybir.AluOpType.mult
    add = mybir.AluOpType.add
    iseq = mybir.AluOpType.is_equal
    aux_t = cpool.tile([NR, 4 + 2 * SEG], F32)
    dma_eng.dma_start(out=aux_t[:], in_=aux[:])
    idx_t = aux_t[:, 0:4]
    iota_t = aux_t[:, 4 : 4 + SEG]
    wbtab_t = aux_t[:, 4 + SEG : 4 + 2 * SEG]

    m_t = cpool.tile([NR, 2], F32)
    nc.vector.tensor_scalar(out=m_t[:, 0:1], in0=idx_t[:, 2:3],
                            scalar1=0.0, scalar2=None, op0=iseq)
    nc.vector.tensor_scalar(out=m_t[:, 1:2], in0=idx_t[:, 2:3],
                            scalar1=1.0, scalar2=None, op0=iseq)
    oh = spool.tile([NR, SEG], F32, tag="oh")
    for a, b, col, mcol in _SEGS:
        nc.vector.tensor_scalar(
            out=oh[:, a:b], in0=iota_t[:, a:b],
            scalar1=idx_t[:, col : col + 1],
            scalar2=m_t[:, mcol : mcol + 1],
            op0=iseq, op1=mult,
        )
    prod = spool.tile([NR, SEG], F32, tag="prod")
    nc.vector.tensor_mul(out=prod[:], in0=oh[:], in1=wbtab_t[:])
    wbp = cpool.tile([NR, 1], F32)
    nc.vector.tensor_reduce(out=wbp[:], in_=prod[:],
                            axis=mybir.AxisListType.X, op=add)
    # transpose [NR,1] -> [1,NR] (tiny SBUF->SBUF DMA), then broadcast to all
    # 128 partitions for use as per-partition scale/bias operands
    wbrow = cpool.tile([1, NR], F32)
    dma_eng.dma_start(out=wbrow[:], in_=wbp[:])
    nc.gpsimd.partition_broadcast(wb_t[:], wbrow[:])


def _gather128(nc, cpool, spool, idx, iotas, wtab, btab, wb_t):
    """Original variant: tables replicated across 128 partitions."""
    mult = mybir.AluOpType.mult
    add = mybir.AluOpType.add
    iseq = mybir.AluOpType.is_equal
    idx_t = cpool.tile([128, 3 * BPC], F32)
    nc.sync.dma_start(out=idx_t[:], in_=idx[:])
    iota_t = cpool.tile([128, SEG], F32)
    nc.sync.dma_start(out=iota_t[:], in_=iotas[:])
    wtab_t = cpool.tile([128, C * SEG], F32)
    nc.sync.dma_start(out=wtab_t[:], in_=wtab[:])
    btab_t = cpool.tile([128, C * SEG], F32)
    nc.sync.dma_start(out=btab_t[:], in_=btab[:])
    m_t = cpool.tile([128, 2 * BPC], F32)
    for i in range(BPC):
        dc = 3 * i + 2
        nc.vector.tensor_scalar(
            out=m_t[:, 2 * i : 2 * i + 1], in0=idx_t[:, dc : dc + 1],
            scalar1=0.0, scalar2=None, op0=iseq,
        )
        nc.vector.tensor_scalar(
            out=m_t[:, 2 * i + 1 : 2 * i + 2], in0=idx_t[:, dc : dc + 1],
            scalar1=1.0, scalar2=None, op0=iseq,
        )
        oh = spool.tile([128, SEG], F32, tag="oh")
        for a, b, col, mcol in _SEGS:
            nc.vector.tensor_scalar(
                out=oh[:, a:b], in0=iota_t[:, a:b],
                scalar1=idx_t[:, 3 * i + col : 3 * i + col + 1],
                scalar2=m_t[:, 2 * i + mcol : 2 * i + mcol + 1],
                op0=iseq, op1=mult,
            )
        for c in range(C):
            for tab_t, off in ((wtab_t, 0), (btab_t, BPC * C)):
                # NOTE: tensor_tensor_reduce wedges this HW/ucode
                # (NRT_EXEC_UNIT_UNRECOVERABLE); use mul + reduce.
                prod = spool.tile([128, SEG], F32, tag="prod")
                nc.vector.tensor_mul(
                    out=prod[:], in0=oh[:],
                    in1=tab_t[:, c * SEG : (c + 1) * SEG],
                )
                nc.vector.tensor_reduce(
                    out=wb_t[:, off + i * C + c : off + i * C + c + 1],
                    in_=prod[:], axis=mybir.AxisListType.X, op=add,
                )


_MODE_DEFAULTS = {
    # mode: (mix, store_eng, bufs) — sim-tuned per IO mode
    "f16": ("alt", "sp", 4),
    "i8f16": ("ddadad", "act", 6),
    "i8i8": ("ddadad", "pool", 6),
}


def _build(reps: int = 1, tile_f=None, bufs: int | None = None,
           mix: str | None = None, gmode: str = "12",
           store_eng: str | None = None, mode: str | None = None):
    """Build the per-core program. reps>1 repeats the streaming stage (used
    only for timing measurements — differencing two rep counts cancels the
    dispatch overhead and one-time costs). mix: 'alt' alternates DVE/ACT for
    the affine, 'dve' uses DVE only, 'act' ACT only. gmode: '12' computes the
    gather on 12 partitions + broadcasts (tiny aux inputs); '128' replicates
    the tables across all partitions. mode: IO precision (see MODE above)."""
    if mode is None:
        mode = MODE
    dmix, dstore, dbufs = _MODE_DEFAULTS[mode]
    if mix is None:
        mix = dmix
    if store_eng is None:
        store_eng = dstore
    if bufs is None:
        bufs = dbufs
    if tile_f is None:
        tile_f = TILE_F
    key = ("nc", reps, tile_f, bufs, mix, gmode, store_eng, mode)
    if key in _CACHE:
        return _CACHE[key]
    in_dt = F16 if mode == "f16" else I8
    out_dt = I8 if mode == "i8i8" else F16
    nc = bacc.Bacc("TRN2", target_bir_lowering=False, debug=False, num_devices=N_CORES)
    NR = 2 * BPC * C  # 12 gathered values: r = off*BPC*C + i*C + c (off: 0=w 1=b)
    img = nc.dram_tensor("img", [BPC, C, H, W], in_dt, kind="ExternalInput").ap()
    if gmode == "12":
        aux = nc.dram_tensor("aux", [NR, 4 + 2 * SEG], F32, kind="ExternalInput").ap()
    else:
        idx = nc.dram_tensor("idx", [128, 3 * BPC], F32, kind="ExternalInput").ap()
        iotas = nc.dram_tensor("iotas", [128, SEG], F32, kind="ExternalInput").ap()
        wtab = nc.dram_tensor("wtab", [128, C * SEG], F32, kind="ExternalInput").ap()
        btab = nc.dram_tensor("btab", [128, C * SEG], F32, kind="ExternalInput").ap()
    if mode != "f16":
        # per-(plane, partition) input dequant scales, one column per plane
        scl = nc.dram_tensor("scl", [128, BPC * C], F32, kind="ExternalInput").ap()
    out = nc.dram_tensor("out", [BPC, C, H, W], out_dt, kind="ExternalOutput").ap()

    mult = mybir.AluOpType.mult
    add = mybir.AluOpType.add
    iseq = mybir.AluOpType.is_equal

    with tile.TileContext(nc) as tc:
        with (
            tc.tile_pool(name="const", bufs=1) as cpool,
            tc.tile_pool(name="scratch", bufs=2) as spool,
            tc.tile_pool(name="io", bufs=bufs) as iopool,
        ):
            # gathered affine params: w at col i*C+c, b at col BPC*C + i*C+c
            wb_t = cpool.tile([128, NR], F32)

            # gather's tiny DMAs ride the SP queue right behind plane-0's load
            # (ACT/pool queues stall them behind slow in-queue waits)
            gather_dma = nc.sync

            def do_gather():
                if gmode == "12":
                    _gather12(nc, cpool, spool, aux, wb_t, NR,
                              dma_eng=gather_dma)
                else:
                    _gather128(nc, cpool, spool, idx, iotas, wtab, btab, wb_t)

            nplanes = BPC * C
            if mode != "f16":
                scl_t = cpool.tile([128, nplanes], F32)
                sc_t = cpool.tile([128, nplanes], F32)   # effective scale
                bq_t = cpool.tile([128, nplanes], F32)   # effective bias (i8i8)
                _orig_gather = do_gather

                def do_gather():
                    _orig_gather()
                    gather_dma.dma_start(out=scl_t[:], in_=scl[:])
                    if mode == "i8f16":
                        # y = (w*s)*q + b
                        nc.vector.tensor_mul(out=sc_t[:], in0=scl_t[:],
                                             in1=wb_t[:, 0:nplanes])
                    else:
                        # z = (w/OUT_SCL)*q + b/(OUT_SCL*s); host dequant
                        # multiplies by OUT_SCL*s
                        nc.vector.tensor_scalar(
                            out=sc_t[:], in0=wb_t[:, 0:nplanes],
                            scalar1=1.0 / OUT_SCL, scalar2=None, op0=mult,
                        )
                        rs_t = cpool.tile([128, nplanes], F32)
                        nc.vector.reciprocal(out=rs_t[:], in_=scl_t[:])
                        nc.vector.tensor_mul(out=bq_t[:],
                                             in0=wb_t[:, nplanes : 2 * nplanes],
                                             in1=rs_t[:])
                        nc.vector.tensor_scalar(
                            out=bq_t[:], in0=bq_t[:],
                            scalar1=1.0 / OUT_SCL, scalar2=None, op0=mult,
                        )

            def plane_sizes(pidx):
                if not isinstance(tile_f, str):
                    return [tile_f] * (PF // tile_f)
                # ramped schedules: smaller tiles at the very start (fast
                # pipeline fill) and very end (fast drain), full planes between
                first, last = {
                    "ramp": ([2048, 2048, 4096], [4096, 2048, 2048]),
                    "ramp2": ([2048, 6144], [6144, 2048]),
                    "ramp3": ([4096, 4096], [4096, 4096]),
                    # chunk the first plane (bounds the gather-transpose DMA's
                    # FIFO delay behind plane loads) and taper the last plane
                    # (short unoverlapped drain)
                    "ramp_d": ([2048, 2048, 2048, 2048],
                               [4096, 2048, 1024, 1024]),
                }[tile_f]
                if pidx == 0:
                    return first
                if pidx == nplanes - 1:
                    return last
                return [PF]

            store = {"sp": nc.sync, "act": nc.scalar, "pool": nc.gpsimd,
                     "pe": nc.tensor}[store_eng]

            def affine(in_ap, out_ap, w_ap, b_ap, k, force_dve=False):
                if mix in ("alt", "dve", "act"):
                    use_dve = mix == "dve" or (mix == "alt" and k % 2 == 0)
                else:
                    # explicit engine pattern, e.g. 'ddadad', cycled per chunk
                    use_dve = mix[(k - 1) % len(mix)] == "d"
                use_dve = use_dve or force_dve
                if use_dve:
                    nc.vector.tensor_scalar(
                        out=out_ap, in0=in_ap,
                        scalar1=w_ap, scalar2=b_ap, op0=mult, op1=add,
                    )
                else:
                    nc.scalar.activation(
                        out=out_ap, in_=in_ap,
                        func=mybir.ActivationFunctionType.Identity,
                        bias=b_ap, scale=w_ap,
                    )

            def w_b(plane):
                p = plane
                if mode == "f16":
                    return (
                        wb_t[:, p : p + 1],
                        wb_t[:, BPC * C + p : BPC * C + p + 1],
                    )
                if mode == "i8i8":
                    return (sc_t[:, p : p + 1], bq_t[:, p : p + 1])
                return (sc_t[:, p : p + 1],
                        wb_t[:, BPC * C + p : BPC * C + p + 1])

            max_tf = PF if isinstance(tile_f, str) else tile_f
            # k starts at 1: the first plane's affine lands on ACT, which is
            # idle while the gather chain occupies DVE
            k = 1
            for _rep in range(reps):
              for plane in range(nplanes):
                i, c = divmod(plane, C)
                src = img[i, c].rearrange("(p r) w -> p (r w)", p=128)
                dst = out[i, c].rearrange("(p r) w -> p (r w)", p=128)
                w_ap, b_ap = w_b(plane)
                pos = 0
                sizes = plane_sizes(plane)
                for ci, sz in enumerate(sizes):
                    tl = iopool.tile([128, max_tf], in_dt, tag="io")
                    nc.sync.dma_start(
                        out=tl[:, :sz], in_=src[:, pos : pos + sz]
                    )
                    if do_gather is not None:
                        # dispatch the gather's tiny DMAs AFTER the first
                        # plane load so the image stream starts immediately
                        do_gather()
                        do_gather = None
                    if in_dt == out_dt:
                        ot = tl
                    else:
                        ot = iopool.tile([128, max_tf], out_dt, tag="io_out")
                    # drain: keep the final chunks' affines on the faster DVE
                    last2 = (_rep == reps - 1 and plane == nplanes - 1
                             and ci >= len(sizes) - 2)
                    affine(tl[:, :sz], ot[:, :sz], w_ap, b_ap, k,
                           force_dve=last2)
                    store.dma_start(
                        out=dst[:, pos : pos + sz], in_=ot[:, :sz]
                    )
                    pos += sz
                    k += 1

    nc.compile()
    _CACHE[key] = nc
    return nc


def make_in_maps(image, camindex, idindex, dataset_type,
                 wcam1, bcam1, wident1, bident1,
                 wcam2, bcam2, wident2, bident2, gmode: str = "12",
                 mode: str | None = None):
    """Host-side sharding + layout: batch-shard the image/indices, replicate
    the tiny tables (pure data movement; all gather math runs on device).
    For int8 modes also computes per-(plane, partition-row-block) scales."""
    if mode is None:
        mode = MODE
    imgf = np.asarray(image, dtype=np.float32)
    if mode == "f16":
        image = np.ascontiguousarray(imgf.astype(np.float16))
        scls = None
    else:
        xb = imgf.reshape(B, C, 128, 8, W)
        mx = np.abs(xb).max(axis=(3, 4))                     # [B,C,128]
        scls = np.maximum(mx, 1e-20).astype(np.float32) / 127.0
        q = np.rint(xb / scls[:, :, :, None, None])
        image = np.ascontiguousarray(
            np.clip(q, -127, 127).astype(np.int8).reshape(B, C, H, W)
        )
    cam = np.asarray(camindex).astype(np.float32)
    idi = np.asarray(idindex).astype(np.float32)
    dts = np.asarray(dataset_type).astype(np.float32)

    iot = np.concatenate(
        [np.arange(NC1), np.arange(NI1), np.arange(NC2), np.arange(NI2)]
    ).astype(np.float32)
    wrow = np.concatenate(
        [np.asarray(t, dtype=np.float32) for t in (wcam1, wident1, wcam2, wident2)],
        axis=0,
    )  # [SEG, 3]
    brow = np.concatenate(
        [np.asarray(t, dtype=np.float32) for t in (bcam1, bident1, bcam2, bident2)],
        axis=0,
    )

    NR = 2 * BPC * C
    in_maps = []
    if gmode == "12":
        # one aux tensor per core: [0:4) idx, [4:4+SEG) iota, [4+SEG:) table
        # row r = off*BPC*C + i*C + c: table (w if off==0 else b), channel c
        aux0 = np.zeros((NR, 4 + 2 * SEG), np.float32)
        aux0[:, 4 : 4 + SEG] = iot
        for r in range(NR):
            off, rem = divmod(r, BPC * C)
            i, c = divmod(rem, C)
            aux0[r, 4 + SEG :] = (wrow if off == 0 else brow)[:, c]
        for k in range(N_CORES):
            s = slice(BPC * k, BPC * (k + 1))
            aux = aux0.copy()
            for r in range(NR):
                off, rem = divmod(r, BPC * C)
                i, c = divmod(rem, C)
                gi = BPC * k + i
                aux[r, 0] = cam[gi]
                aux[r, 1] = idi[gi]
                aux[r, 2] = dts[gi]
            in_maps.append({"img": image[s], "aux": aux})
    else:
        iotas = np.ascontiguousarray(np.broadcast_to(iot, (128, SEG)))
        wtab = np.ascontiguousarray(
            np.broadcast_to(wrow.T.reshape(-1), (128, C * SEG))
        )
        btab = np.ascontiguousarray(
            np.broadcast_to(brow.T.reshape(-1), (128, C * SEG))
        )
        for k in range(N_CORES):
            s = slice(BPC * k, BPC * (k + 1))
            row = np.stack([cam[s], idi[s], dts[s]], axis=1).reshape(-1)
            idx = np.ascontiguousarray(np.broadcast_to(row, (128, 3 * BPC)))
            in_maps.append(
                {"img": image[s], "idx": idx, "iotas": iotas,
                 "wtab": wtab, "btab": btab}
            )
    if scls is not None:
        for k in range(N_CORES):
            s = slice(BPC * k, BPC * (k + 1))
            # [BPC,C,128] -> [128, BPC*C] with column p = i*C + c
            in_maps[k]["scl"] = np.ascontiguousarray(
                scls[s].reshape(BPC * C, 128).T
            )
    return in_maps


def kernel(image, camindex, idindex, dataset_type,
           wcam1, bcam1, wident1, bident1,
           wcam2, bcam2, wident2, bident2) -> np.ndarray:
    nc = _build()
    in_maps = make_in_maps(
        image, camindex, idindex, dataset_type,
        wcam1, bcam1, wident1, bident1, wcam2, bcam2, wident2, bident2,
    )
    res = bass_utils.run_bass_kernel_spmd(nc, in_maps, list(range(N_CORES)))
    outs = []
    for k in range(N_CORES):
        o = np.asarray(res.results[k]["out"])
        if MODE == "i8i8":
            # dequant: per-(plane, partition-row-block) scale, OUT_SCL headroom
            so = OUT_SCL * in_maps[k]["scl"].T.reshape(BPC, C, 128)
            o = (o.reshape(BPC, C, 128, 8, W).astype(np.float32)
                 * so[:, :, :, None, None]).reshape(BPC, C, H, W)
        else:
            o = o.astype(np.float32)
        outs.append(o)
    return np.concatenate(outs, axis=0)

